# revision 16
# baseline (speedup 1.0000x reference)
# Multi-head attention (N=4, S=2048, E=512, H=8, D=64) on 8 NeuronCores.
#
# Sharding: core c -> (batch n = c//2, query half qh = c%2). Each core
# computes attention for its 1024 query rows against the full 2048 keys of
# its batch, all 8 heads, and the full output projection for its rows, so
# outputs are disjoint and no collectives are needed.
#
# Host-side weight folding (pure weight algebra, done once):
#   A  = Wq^T @ Wk / 8          scores = (Xq @ A) @ Xk^T  (raw K, one proj)
#   M_h = Wv^T @ Wo[:, h]^T     out += (attn @ Xv_h) @ M_h (Wv applied post)
#   btot = bo + Wo @ tile(bv,8) exact because attention rows sum to 1
#   bk-term cancels in softmax (constant over k); bq-term handled via a
#   per-k bias correction (zero for this problem's inputs).
#
# Device per core:
#   - PE-transpose Q,K chunks (bf16) to [e, s] layout
#   - XqA^T = A^T-proj of Q^T per head
#   - scores^T[k,q] = K_h^T.T @ XqA_h^T  (PSUM f32)
#   - exp on ACT straight from PSUM with per-partition mask bias -> bf16
#   - AV^T = V_aug.T @ exp^T with a ones column in V giving softmax
#     denominators as row 64; normalize with DVE + DMA-replicated recip
#   - out[q,:] = sum_h AVT_h.T @ M_h (+btot), accumulated in PSUM

import numpy as np
import ml_dtypes

import concourse.bass as bass
import concourse.tile as tile
from concourse import bacc, mybir
from concourse.bass_utils import run_bass_kernel_spmd
from concourse.masks import make_identity

F32 = mybir.dt.float32
BF16 = mybir.dt.bfloat16
I32 = mybir.dt.int32

H = 8
D = 64
E = 512
N_CORES = 8
FULL_N, FULL_S = 4, 2048
SQ, SK = 1024, 2048  # per-core query rows / key rows
MASK_BIAS = -1.25e8  # == -1e9 / sqrt(64), applied pre-softmax


def _emit(tc, t, SQ, SK, has_qbias, stop_phase=99):
    nc = tc.nc
    NQC = SQ // 128           # query chunks (transpose granularity)
    NKC = SK // 128           # key chunks
    QGS = min(512, SQ)        # q group size for matmul free dim
    NQG = SQ // QGS
    sub, mult, add = (mybir.AluOpType.subtract, mybir.AluOpType.mult,
                      mybir.AluOpType.add)

    with (
        tc.tile_pool(name="singles", bufs=1) as singles,
        tc.tile_pool(name="stage", bufs=3) as stage,
        tc.tile_pool(name="expp", bufs=2) as expp,
        tc.tile_pool(name="small", bufs=3) as small,
        tc.tile_pool(name="outp", bufs=2) as outp,
        tc.tile_pool(name="p_sc", bufs=2, space="PSUM") as p_sc,
        tc.tile_pool(name="p_misc", bufs=4, space="PSUM") as p_misc,
    ):
        # ---- constants / weights ----
        ident = singles.tile([128, 128], BF16)
        make_identity(nc, ident)

        a_sb = singles.tile([128, D], BF16)
        nc.sync.dma_start(a_sb, t["a2"][:])
        m_sb = singles.tile([64, H, E], BF16)
        nc.sync.dma_start(m_sb, t["m2"][:])
        btot_rep = singles.tile([128, E], F32)
        nc.gpsimd.dma_start(btot_rep, t["btot"][:][None, :].to_broadcast([128, E]))

        # mask -> additive bias, [128, NKC] with k = kt*128 + p
        mask_i = singles.tile([128, NKC], I32)
        nc.gpsimd.dma_start(mask_i, t["mask"][:].rearrange("(kt p) -> p kt", p=128))
        mask_f = singles.tile([128, NKC], F32)
        nc.vector.tensor_copy(mask_f, mask_i)
        mbias = singles.tile([128, NKC], F32)
        # (mask - 1) * (-MASK_BIAS):  mask=0 -> MASK_BIAS, mask=1 -> 0
        nc.vector.tensor_scalar(out=mbias, in0=mask_f, scalar1=1.0,
                                scalar2=-MASK_BIAS, op0=sub, op1=mult)

        # ---- persistent tensors ----
        ones1 = singles.tile([1, D], F32)        # rank-1 broadcast helper
        nc.vector.memset(ones1, 1.0)
        qt = singles.tile([128, 4, SQ], BF16)    # query^T: e=fc*128+p
        kt = singles.tile([128, 4, SK], BF16)    # key^T
        vt = singles.tile([128, NKC, H, D + 1], BF16)  # value + ones col
        xqa = singles.tile([128, 4, SQ], BF16)   # (Xq @ A)^T per head
        avt = singles.tile([65, H, SQ], BF16)    # normalized (attn @ V)^T
        nc.vector.memset(vt[:, :, :, D:D + 1], 1.0)

        # ---- load + cast + PE-transpose Q and K ----
        def load_transposed(src, dst, nchunks, tag):
            for c in range(nchunks):
                raw = stage.tile([128, E], F32, tag="ld")
                nc.sync.dma_start(raw, src[c * 128:(c + 1) * 128, :])
                cast = stage.tile([128, E], BF16, tag="cast")
                nc.gpsimd.tensor_copy(cast, raw)
                tp = p_misc.tile([128, 4, 128], BF16, tag="ps")
                for eg in range(4):
                    nc.tensor.transpose(tp[:, eg, :],
                                        cast[:, eg * 128:(eg + 1) * 128], ident)
                nc.vector.tensor_copy(dst[:, :, c * 128:(c + 1) * 128], tp)

        load_transposed(t["query"][:], qt, NQC, "q")
        load_transposed(t["key"][:], kt, NKC, "k")
        if stop_phase <= 1:
            ob0 = outp.tile([128, E], F32, tag="ob")
            nc.vector.memset(ob0, 0.0)
            nc.vector.tensor_copy(ob0[:, 0:QGS], qt[:, 0, 0:QGS])
            nc.sync.dma_start(t["out"][0:128, :], ob0)
            return

        # ---- load + cast V (natural layout, per-head blocks + ones col) ----
        for c in range(NKC):
            raw = stage.tile([128, E], F32, tag="ld")
            nc.sync.dma_start(raw, t["value"][c * 128:(c + 1) * 128, :])
            nc.gpsimd.tensor_copy(vt[:, c, :, 0:D],
                                  raw.rearrange("p (h d) -> p h d", h=H))

        # ---- optional exact bq correction: per-(h,k) additive bias ----
        # scores^T gains (Xk_h @ (Wk^T bq / 8))[k], constant over q.
        hbias = []
        if has_qbias:
            w2 = singles.tile([128, 1], BF16)
            nc.sync.dma_start(w2, t["w2"][:])
            for h in range(H):
                bp, fc = 64 * (h % 2), h // 2
                row = small.tile([1, SK], F32, tag="hb_row")
                for g in range(SK // 512):
                    ps = p_misc.tile([128, 512], F32, tag="ps")
                    nc.tensor.matmul(ps[0:1, :], lhsT=w2[bp:bp + 64, :],
                                     rhs=kt[bp:bp + 64, fc, g * 512:(g + 1) * 512],
                                     start=True, stop=True)
                    nc.vector.tensor_copy(row[:, g * 512:(g + 1) * 512],
                                          ps[0:1, :])
                hb = singles.tile([128, NKC], F32, name=f"hbias{h}")
                nc.gpsimd.dma_start(hb, row[0, :].rearrange("(kt p) -> p kt", p=128))
                nc.vector.tensor_tensor(out=hb, in0=hb, in1=mbias, op=add)
                hbias.append(hb)
        else:
            hbias = [mbias] * H

        # ---- XqA^T projection per head ----
        for h in range(H):
            bp, fc = 64 * (h % 2), h // 2
            for g in range(NQG):
                ps = p_misc.tile([128, QGS], F32, tag="ps")
                nc.tensor.matmul(ps[bp:bp + 64, :], lhsT=a_sb[bp:bp + 64, :],
                                 rhs=qt[bp:bp + 64, fc, g * QGS:(g + 1) * QGS],
                                 start=True, stop=True)
                nc.vector.tensor_copy(xqa[bp:bp + 64, fc, g * QGS:(g + 1) * QGS],
                                      ps[bp:bp + 64, :])

        if stop_phase <= 2:
            ob1 = outp.tile([128, E], F32, tag="ob")
            nc.vector.memset(ob1, 0.0)
            nc.vector.tensor_copy(ob1[:, 0:QGS], xqa[:, 0, 0:QGS])
            nc.sync.dma_start(t["out"][0:128, :], ob1)
            return

        # ---- per-head: scores^T -> exp -> AV^T (software-pipelined) ----
        def emit_scores_exp(h):
            bp, fc = 64 * (h % 2), h // 2
            ex = expp.tile([128, NKC, SQ], BF16, tag="exp")
            for kt_i in range(NKC):
                ps = p_sc.tile([128, SQ], F32, tag="sc")
                for g in range(NQG):
                    nc.tensor.matmul(
                        ps[:, g * QGS:(g + 1) * QGS],
                        lhsT=kt[bp:bp + 64, fc, kt_i * 128:(kt_i + 1) * 128],
                        rhs=xqa[bp:bp + 64, fc, g * QGS:(g + 1) * QGS],
                        start=True, stop=True)
                nc.scalar.activation(out=ex[:, kt_i, :], in_=ps,
                                     func=mybir.ActivationFunctionType.Exp,
                                     bias=hbias[h][:, kt_i:kt_i + 1], scale=1.0)
            return ex

        def emit_av(h, ex, normalize=True):
            for g in range(NQG):
                ps = p_misc.tile([128, QGS], F32, tag="ps")
                for c in range(NKC):
                    nc.tensor.matmul(ps[0:D + 1, :], lhsT=vt[:, c, h, :],
                                     rhs=ex[:, c, g * QGS:(g + 1) * QGS],
                                     start=(c == 0), stop=(c == NKC - 1))
                if not normalize:
                    nc.vector.tensor_copy(avt[0:D, h, g * QGS:(g + 1) * QGS],
                                          ps[0:D, :])
                    continue
                # Partition-broadcast of the denominator row via a rank-1
                # matmul (ones[64] (x) denom[q]), then recip + multiply.
                den = small.tile([1, QGS], F32, tag="den")
                nc.vector.tensor_copy(den, ps[D:D + 1, :])
                pb = p_misc.tile([128, QGS], F32, tag="ps")
                nc.tensor.matmul(pb[0:D, :], lhsT=ones1, rhs=den,
                                 start=True, stop=True)
                rep = small.tile([64, QGS], F32, tag="rep")
                nc.vector.reciprocal(rep, pb[0:D, :])
                nc.vector.tensor_tensor(out=avt[0:D, h, g * QGS:(g + 1) * QGS],
                                        in0=ps[0:D, :], in1=rep, op=mult)

        if stop_phase <= 3:
            ex0 = emit_scores_exp(0)
            ob2 = outp.tile([128, E], F32, tag="ob")
            nc.vector.memset(ob2, 0.0)
            nc.vector.tensor_copy(ob2[:, 0:QGS], ex0[:, 0, 0:QGS])
            nc.sync.dma_start(t["out"][0:128, :], ob2)
            return

        if stop_phase <= 5:
            ex0 = emit_scores_exp(0)
            emit_av(0, ex0, normalize=(stop_phase == 5))
            ob3 = outp.tile([128, E], F32, tag="ob")
            nc.vector.memset(ob3, 0.0)
            nc.vector.tensor_copy(ob3[0:D, 0:QGS], avt[0:D, 0, 0:QGS])
            nc.sync.dma_start(t["out"][0:128, :], ob3)
            return

        prev = None
        for h in range(H):
            ex = emit_scores_exp(h)
            if prev is not None:
                emit_av(h - 1, prev)
            prev = ex
        emit_av(H - 1, prev)

        # ---- output projection: out[q,:] = sum_h AVT_h^T @ M_h + btot ----
        for q_i in range(SQ // 128):
            ps = p_misc.tile([128, E], F32, tag="ps")
            for h in range(H):
                nc.tensor.matmul(ps, lhsT=avt[0:D, h, q_i * 128:(q_i + 1) * 128],
                                 rhs=m_sb[0:D, h, :],
                                 start=(h == 0), stop=(h == H - 1))
            ob = outp.tile([128, E], F32, tag="ob")
            nc.vector.tensor_tensor(out=ob, in0=ps, in1=btot_rep, op=add)
            nc.sync.dma_start(t["out"][q_i * 128:(q_i + 1) * 128, :], ob)


def build_module(SQ=SQ, SK=SK, has_qbias=False, stop_phase=99):
    nc = bacc.Bacc()
    t = {
        "query": nc.dram_tensor("query", [SQ, E], F32, kind="ExternalInput"),
        "key": nc.dram_tensor("key", [SK, E], F32, kind="ExternalInput"),
        "value": nc.dram_tensor("value", [SK, E], F32, kind="ExternalInput"),
        "mask": nc.dram_tensor("mask", [SK], I32, kind="ExternalInput"),
        "a2": nc.dram_tensor("a2", [128, D], BF16, kind="ExternalInput"),
        "m2": nc.dram_tensor("m2", [64, H, E], BF16, kind="ExternalInput"),
        "btot": nc.dram_tensor("btot", [E], F32, kind="ExternalInput"),
        "out": nc.dram_tensor("out", [SQ, E], F32, kind="ExternalOutput"),
    }
    if has_qbias:
        t["w2"] = nc.dram_tensor("w2", [128, 1], BF16, kind="ExternalInput")
    with tile.TileContext(nc) as tc:
        _emit(tc, t, SQ, SK, has_qbias, stop_phase)
    nc.compile()
    return nc


_MODULE_CACHE = {}


def _get_module(SQ, SK, has_qbias):
    key = (SQ, SK, has_qbias)
    if key not in _MODULE_CACHE:
        _MODULE_CACHE[key] = build_module(SQ, SK, has_qbias)
    return _MODULE_CACHE[key]


def _fold_weights(Wq, Wk, Wv, Wo, bv, bo):
    Wq, Wk, Wv, Wo = (np.asarray(w, np.float64) for w in (Wq, Wk, Wv, Wo))
    A = (Wq.T @ Wk) / np.sqrt(np.float64(D))
    a2 = np.concatenate([A, A], axis=0).astype(ml_dtypes.bfloat16)  # [128, 64]
    Ms = [Wv.T @ Wo[:, h * D:(h + 1) * D].T for h in range(H)]
    m2 = np.stack(Ms, axis=1).astype(ml_dtypes.bfloat16)  # [64, H, E]
    btot = (np.asarray(bo, np.float64)
            + Wo @ np.tile(np.asarray(bv, np.float64), H)).astype(np.float32)
    return a2, m2, btot


def _run(inputs, trace=False):
    query = np.asarray(inputs["query"], np.float32)
    key = np.asarray(inputs["key"], np.float32)
    value = np.asarray(inputs["value"], np.float32)
    mask = np.asarray(inputs["mask"])
    a2, m2, btot = _fold_weights(inputs["Wq"], inputs["Wk"], inputs["Wv"],
                                 inputs["Wo"], inputs["bv"], inputs["bo"])
    bq = np.asarray(inputs["bq"], np.float64)
    bk = np.asarray(inputs["bk"], np.float64)  # noqa: F841  (cancels in softmax)
    has_qbias = bool(np.any(bq != 0))
    w2 = None
    if has_qbias:
        w2v = (np.asarray(inputs["Wk"], np.float64).T @ bq) / np.sqrt(float(D))
        w2 = np.concatenate([w2v, w2v]).reshape(128, 1).astype(ml_dtypes.bfloat16)

    n_batch, S = query.shape[0], query.shape[1]
    sq = S // 2
    nc = _get_module(sq, S, has_qbias)

    in_maps = []
    for c in range(N_CORES):
        n, qh = divmod(c, 2)
        m = {
            "query": np.ascontiguousarray(query[n, qh * sq:(qh + 1) * sq, :]),
            "key": np.ascontiguousarray(key[n]),
            "value": np.ascontiguousarray(value[n]),
            "mask": np.ascontiguousarray(mask[n, 0, 0, :].astype(np.int32)),
            "a2": a2, "m2": m2, "btot": btot,
        }
        if has_qbias:
            m["w2"] = w2
        in_maps.append(m)

    res = run_bass_kernel_spmd(nc, in_maps, core_ids=list(range(N_CORES)),
                               trace=trace)
    out = np.empty((n_batch, S, E), np.float32)
    for c, r in enumerate(res.results):
        n, qh = divmod(c, 2)
        out[n, qh * sq:(qh + 1) * sq, :] = r["out"]
    return out, res


def kernel(**inputs) -> np.ndarray:
    out, _ = _run(inputs, trace=False)
    return out
